# revision 8
# baseline (speedup 1.0000x reference)
"""Trainium2 Bass kernel for nn_MessageFunction (GNN message passing).

Computes, for each batch b:
    out[b] = W_e @ e_vw[b] + W_h @ h_w[b] + (b_e + b_h)[:, None]

Shapes: e_vw/h_w: [B=1024, 128, N=512] f32, W_e/W_h: [128, 128], out: [B, 128, 512].
h_v is an unused input (the reference never reads it) — never transferred.

Strategy: data-parallel over B across 8 cores (128 batches/core). Per batch,
two accumulating 128x128 @ 128x512 matmuls into one PSUM bank, bias folded
into the PSUM->SBUF copy. Memory-bound: per core 64MB in + 32MB out.
Batches are grouped G at a time per DMA (G*256KB per transfer) for bandwidth.
"""

import numpy as np

import concourse.bass as bass
import concourse.mybir as mybir
import concourse.tile as tile
from concourse import bacc
from concourse.bass_utils import run_bass_kernel_spmd

import os as _os

B, E, NODE, M, N = 1024, 128, 128, 128, 512
N_CORES = 8
B_SH = B // N_CORES  # 128 batches per core
G = int(_os.environ.get("K_G", "8"))  # batches per DMA group
G_MM = int(_os.environ.get("K_GMM", "4"))  # matmul/psum subgroup size
IO_BUFS = int(_os.environ.get("K_BUFS", "3"))
USE_F32R = _os.environ.get("K_F32R", "0") == "1"
F32 = mybir.dt.float32
F32R = mybir.dt.float32r

_cache = {}


def _build():
    nc = bacc.Bacc(None, target_bir_lowering=False)
    e = nc.dram_tensor("e", [B_SH, E, N], F32, kind="ExternalInput")
    h = nc.dram_tensor("h", [B_SH, NODE, N], F32, kind="ExternalInput")
    w_eT = nc.dram_tensor("w_eT", [E, M], F32, kind="ExternalInput")
    w_hT = nc.dram_tensor("w_hT", [NODE, M], F32, kind="ExternalInput")
    bias = nc.dram_tensor("bias", [M, 1], F32, kind="ExternalInput")
    out = nc.dram_tensor("out", [B_SH, M, N], F32, kind="ExternalOutput")

    with tile.TileContext(nc) as tc:
        with (
            tc.tile_pool(name="consts", bufs=1) as consts,
            tc.tile_pool(name="io", bufs=IO_BUFS) as io,
            tc.tile_pool(name="psum", bufs=8, space="PSUM") as psum_pool,
        ):
            wE = consts.tile([E, M], F32)
            nc.sync.dma_start(wE[:], w_eT[:])
            wH = consts.tile([NODE, M], F32)
            nc.sync.dma_start(wH[:], w_hT[:])
            bias_t = consts.tile([M, 1], F32)
            nc.sync.dma_start(bias_t[:], bias[:])

            cast = (lambda ap: ap.bitcast(F32R)) if USE_F32R else (lambda ap: ap)
            for g in range(B_SH // G):
                sl = slice(g * G, (g + 1) * G)
                et = io.tile([E, G, N], F32, tag="e")
                ht = io.tile([NODE, G, N], F32, tag="h")
                ot = io.tile([M, G, N], F32, tag="o")
                nc.sync.dma_start(et[:], e[sl].rearrange("b p n -> p b n"))
                nc.sync.dma_start(ht[:], h[sl].rearrange("b p n -> p b n"))
                for jj in range(0, G, G_MM):
                    pss = [
                        psum_pool.tile([M, N], F32, tag="ps", name="ps")
                        for _ in range(G_MM)
                    ]
                    # weight-grouped: G_MM consecutive MMs share the
                    # stationary operand, so LDWEIGHTS overlaps cleanly
                    for i, ps in enumerate(pss):
                        nc.tensor.matmul(
                            ps[:], cast(wE[:]), cast(et[:, jj + i]),
                            start=True, stop=False,
                        )
                    for i, ps in enumerate(pss):
                        nc.tensor.matmul(
                            ps[:], cast(wH[:]), cast(ht[:, jj + i]),
                            start=False, stop=True,
                        )
                    for i, ps in enumerate(pss):
                        nc.vector.tensor_scalar_add(
                            ot[:, jj + i], ps[:], bias_t[:]
                        )
                nc.sync.dma_start(out[sl].rearrange("b p n -> p b n"), ot[:])

    nc.compile()
    return nc


def _get_nc():
    if "nc" not in _cache:
        _cache["nc"] = _build()
    return _cache["nc"]


def kernel(h_v, h_w, e_vw, W_e, b_e, W_h, b_h, **_ignored):
    h_w = np.ascontiguousarray(np.asarray(h_w, dtype=np.float32))
    e_vw = np.ascontiguousarray(np.asarray(e_vw, dtype=np.float32))
    w_eT = np.ascontiguousarray(np.asarray(W_e, dtype=np.float32).T)
    w_hT = np.ascontiguousarray(np.asarray(W_h, dtype=np.float32).T)
    bias = (
        np.asarray(b_e, dtype=np.float32) + np.asarray(b_h, dtype=np.float32)
    ).reshape(M, 1)

    nc = _get_nc()
    in_maps = []
    for c in range(N_CORES):
        sl = slice(c * B_SH, (c + 1) * B_SH)
        in_maps.append(
            {
                "e": e_vw[sl],
                "h": h_w[sl],
                "w_eT": w_eT,
                "w_hT": w_hT,
                "bias": bias,
            }
        )
    res = run_bass_kernel_spmd(nc, in_maps, core_ids=list(range(N_CORES)))
    return np.concatenate([r["out"] for r in res.results], axis=0)


# revision 9
# speedup vs baseline: 1.3344x; 1.3344x over previous
"""Trainium2 Bass kernel for nn_MessageFunction (GNN message passing).

Computes, for each batch b:
    out[b] = W_e @ e_vw[b] + W_h @ h_w[b] + (b_e + b_h)[:, None]

Shapes: e_vw/h_w: [B=1024, 128, N=512] f32, W_e/W_h: [128, 128], out: [B, 128, 512].
h_v is an unused input (the reference never reads it) — never transferred.

Strategy: data-parallel over B across 8 cores (128 batches/core). Per batch,
two accumulating 128x128 @ 128x512 matmuls into one PSUM bank, bias folded
into the PSUM->SBUF copy. Memory-bound: per core 64MB in + 32MB out.
Batches are grouped G at a time per DMA (G*256KB per transfer) for bandwidth.
"""

import numpy as np

import concourse.bass as bass
import concourse.mybir as mybir
import concourse.tile as tile
from concourse import bacc
from concourse.bass_utils import run_bass_kernel_spmd

import os as _os

B, E, NODE, M, N = 1024, 128, 128, 128, 512
N_CORES = 8
B_SH = B // N_CORES  # 128 batches per core
G = int(_os.environ.get("K_G", "8"))  # batches per DMA group
G_MM = int(_os.environ.get("K_GMM", "4"))  # matmul/psum subgroup size
IO_BUFS = int(_os.environ.get("K_BUFS", "3"))
USE_F32R = _os.environ.get("K_F32R", "0") == "1"
F32 = mybir.dt.float32
F32R = mybir.dt.float32r

_cache = {}


def _build():
    nc = bacc.Bacc(None, target_bir_lowering=False)
    e = nc.dram_tensor("e", [B_SH, E, N], F32, kind="ExternalInput")
    h = nc.dram_tensor("h", [B_SH, NODE, N], F32, kind="ExternalInput")
    w_eT = nc.dram_tensor("w_eT", [E, M], F32, kind="ExternalInput")
    w_hT = nc.dram_tensor("w_hT", [NODE, M], F32, kind="ExternalInput")
    bias = nc.dram_tensor("bias", [M, 1], F32, kind="ExternalInput")
    out = nc.dram_tensor("out", [B_SH, M, N], F32, kind="ExternalOutput")

    with tile.TileContext(nc) as tc:
        with (
            tc.tile_pool(name="consts", bufs=1) as consts,
            tc.tile_pool(name="io", bufs=IO_BUFS) as io,
            tc.tile_pool(name="psum", bufs=8, space="PSUM") as psum_pool,
        ):
            wE = consts.tile([E, M], F32)
            nc.sync.dma_start(wE[:], w_eT[:])
            wH = consts.tile([NODE, M], F32)
            nc.sync.dma_start(wH[:], w_hT[:])
            bias_t = consts.tile([M, 1], F32)
            nc.sync.dma_start(bias_t[:], bias[:])

            cast = (lambda ap: ap.bitcast(F32R)) if USE_F32R else (lambda ap: ap)
            for g in range(B_SH // G):
                sl = slice(g * G, (g + 1) * G)
                et = io.tile([E, G, N], F32, tag="e")
                ht = io.tile([NODE, G, N], F32, tag="h")
                ot = io.tile([M, G, N], F32, tag="o")
                nc.sync.dma_start(et[:], e[sl].rearrange("b p n -> p b n"))
                nc.sync.dma_start(ht[:], h[sl].rearrange("b p n -> p b n"))
                for jj in range(0, G, G_MM):
                    pss = [
                        psum_pool.tile([M, N], F32, tag="ps", name="ps")
                        for _ in range(G_MM)
                    ]
                    # weight-grouped: G_MM consecutive MMs share the
                    # stationary operand, so LDWEIGHTS overlaps cleanly
                    for i, ps in enumerate(pss):
                        nc.tensor.matmul(
                            ps[:], cast(wE[:]), cast(et[:, jj + i]),
                            start=True, stop=False,
                        )
                    for i, ps in enumerate(pss):
                        nc.tensor.matmul(
                            ps[:], cast(wH[:]), cast(ht[:, jj + i]),
                            start=False, stop=True,
                        )
                    for i, ps in enumerate(pss):
                        nc.vector.tensor_scalar_add(
                            ot[:, jj + i], ps[:], bias_t[:]
                        )
                out_eng = nc.scalar if _os.environ.get("K_OUTSCALAR") == "1" else nc.sync
                out_eng.dma_start(out[sl].rearrange("b p n -> p b n"), ot[:])

    nc.compile()
    return nc


def _get_nc():
    if "nc" not in _cache:
        _cache["nc"] = _build()
    return _cache["nc"]


def kernel(h_v, h_w, e_vw, W_e, b_e, W_h, b_h, **_ignored):
    h_w = np.ascontiguousarray(np.asarray(h_w, dtype=np.float32))
    e_vw = np.ascontiguousarray(np.asarray(e_vw, dtype=np.float32))
    w_eT = np.ascontiguousarray(np.asarray(W_e, dtype=np.float32).T)
    w_hT = np.ascontiguousarray(np.asarray(W_h, dtype=np.float32).T)
    bias = (
        np.asarray(b_e, dtype=np.float32) + np.asarray(b_h, dtype=np.float32)
    ).reshape(M, 1)

    nc = _get_nc()
    in_maps = []
    for c in range(N_CORES):
        sl = slice(c * B_SH, (c + 1) * B_SH)
        in_maps.append(
            {
                "e": e_vw[sl],
                "h": h_w[sl],
                "w_eT": w_eT,
                "w_hT": w_hT,
                "bias": bias,
            }
        )
    res = run_bass_kernel_spmd(nc, in_maps, core_ids=list(range(N_CORES)))
    return np.concatenate([r["out"] for r in res.results], axis=0)


# revision 10
# speedup vs baseline: 1.3648x; 1.0227x over previous
"""Trainium2 Bass kernel for nn_MessageFunction (GNN message passing).

Computes, for each batch b:
    out[b] = W_e @ e_vw[b] + W_h @ h_w[b] + (b_e + b_h)[:, None]

Shapes: e_vw/h_w: [B=1024, 128, N=512] f32, W_e/W_h: [128, 128], out: [B, 128, 512].
h_v is an unused input (the reference never reads it) — never transferred.

Strategy: data-parallel over B across 8 cores (128 batches/core). Per batch,
two accumulating 128x128 @ 128x512 matmuls into one PSUM bank, bias folded
into the PSUM->SBUF copy. Memory-bound: per core 64MB in + 32MB out.
Batches are grouped G at a time per DMA (G*256KB per transfer) for bandwidth.
"""

import numpy as np

import concourse.bass as bass
import concourse.mybir as mybir
import concourse.tile as tile
from concourse import bacc
from concourse.bass_utils import run_bass_kernel_spmd

import os as _os

B, E, NODE, M, N = 1024, 128, 128, 128, 512
N_CORES = 8
B_SH = B // N_CORES  # 128 batches per core
G = int(_os.environ.get("K_G", "8"))  # batches per DMA group
G_MM = int(_os.environ.get("K_GMM", "4"))  # matmul/psum subgroup size
IO_BUFS = int(_os.environ.get("K_BUFS", "3"))
USE_F32R = _os.environ.get("K_F32R", "0") == "1"
F32 = mybir.dt.float32
F32R = mybir.dt.float32r

_cache = {}


def _build():
    nc = bacc.Bacc(None, target_bir_lowering=False)
    e = nc.dram_tensor("e", [B_SH, E, N], F32, kind="ExternalInput")
    h = nc.dram_tensor("h", [B_SH, NODE, N], F32, kind="ExternalInput")
    w_eT = nc.dram_tensor("w_eT", [E, M], F32, kind="ExternalInput")
    w_hT = nc.dram_tensor("w_hT", [NODE, M], F32, kind="ExternalInput")
    bias = nc.dram_tensor("bias", [M, 1], F32, kind="ExternalInput")
    out = nc.dram_tensor("out", [B_SH, M, N], F32, kind="ExternalOutput")

    with tile.TileContext(nc) as tc:
        with (
            tc.tile_pool(name="consts", bufs=1) as consts,
            tc.tile_pool(name="io", bufs=IO_BUFS) as io,
            tc.tile_pool(name="psum", bufs=8, space="PSUM") as psum_pool,
        ):
            wE = consts.tile([E, M], F32)
            nc.sync.dma_start(wE[:], w_eT[:])
            wH = consts.tile([NODE, M], F32)
            nc.sync.dma_start(wH[:], w_hT[:])
            bias_t = consts.tile([M, 1], F32)
            nc.sync.dma_start(bias_t[:], bias[:])

            cast = (lambda ap: ap.bitcast(F32R)) if USE_F32R else (lambda ap: ap)
            for g in range(B_SH // G):
                sl = slice(g * G, (g + 1) * G)
                et = io.tile([E, G, N], F32, tag="e")
                ht = io.tile([NODE, G, N], F32, tag="h")
                ot = io.tile([M, G, N], F32, tag="o")
                nc.sync.dma_start(et[:], e[sl].rearrange("b p n -> p b n"))
                nc.sync.dma_start(ht[:], h[sl].rearrange("b p n -> p b n"))
                for jj in range(0, G, G_MM):
                    pss = [
                        psum_pool.tile([M, N], F32, tag="ps", name="ps")
                        for _ in range(G_MM)
                    ]
                    # weight-grouped: G_MM consecutive MMs share the
                    # stationary operand, so LDWEIGHTS overlaps cleanly
                    for i, ps in enumerate(pss):
                        nc.tensor.matmul(
                            ps[:], cast(wE[:]), cast(et[:, jj + i]),
                            start=True, stop=False,
                        )
                    for i, ps in enumerate(pss):
                        nc.tensor.matmul(
                            ps[:], cast(wH[:]), cast(ht[:, jj + i]),
                            start=False, stop=True,
                        )
                    for i, ps in enumerate(pss):
                        nc.vector.tensor_scalar_add(
                            ot[:, jj + i], ps[:], bias_t[:]
                        )
                out_eng = nc.scalar if _os.environ.get("K_OUTSCALAR") == "1" else nc.sync
                if _os.environ.get("K_OUTSPLIT") == "1" and G >= 2:
                    hg = G // 2
                    out_eng.dma_start(
                        out[g * G : g * G + hg].rearrange("b p n -> p b n"),
                        ot[:, :hg],
                    )
                    out_eng.dma_start(
                        out[g * G + hg : (g + 1) * G].rearrange("b p n -> p b n"),
                        ot[:, hg:],
                    )
                else:
                    out_eng.dma_start(out[sl].rearrange("b p n -> p b n"), ot[:])

    nc.compile()
    return nc


def _get_nc():
    if "nc" not in _cache:
        _cache["nc"] = _build()
    return _cache["nc"]


def kernel(h_v, h_w, e_vw, W_e, b_e, W_h, b_h, **_ignored):
    h_w = np.ascontiguousarray(np.asarray(h_w, dtype=np.float32))
    e_vw = np.ascontiguousarray(np.asarray(e_vw, dtype=np.float32))
    w_eT = np.ascontiguousarray(np.asarray(W_e, dtype=np.float32).T)
    w_hT = np.ascontiguousarray(np.asarray(W_h, dtype=np.float32).T)
    bias = (
        np.asarray(b_e, dtype=np.float32) + np.asarray(b_h, dtype=np.float32)
    ).reshape(M, 1)

    nc = _get_nc()
    in_maps = []
    for c in range(N_CORES):
        sl = slice(c * B_SH, (c + 1) * B_SH)
        in_maps.append(
            {
                "e": e_vw[sl],
                "h": h_w[sl],
                "w_eT": w_eT,
                "w_hT": w_hT,
                "bias": bias,
            }
        )
    res = run_bass_kernel_spmd(nc, in_maps, core_ids=list(range(N_CORES)))
    return np.concatenate([r["out"] for r in res.results], axis=0)
